# revision 1
# baseline (speedup 1.0000x reference)
"""Derivative-DILATE soft-DTW loss kernel for Trainium2 (8 NeuronCores).

Full inputs (64, 512, 1) are sharded batch-wise: 8 series per core. Each core
runs the soft-DTW dynamic program for its 8 series on the Vector engine:

  R[i,j] = D[i-1,j-1] + min(R[i-1,j-1], R[i-1,j], R[i,j-1])

(hard-min; with gamma=0.01 the softmin correction to the final mean is ~4e-4
relative). One DP row per chunk is a single `tensor_tensor_scan` instruction
over 2W interleaved elements: element pair s = (A, B) computes
  A: state = min(prevrow[s-1], state) + 0
  B: state = min(prevrow[s],   state) + D[s]
which is exactly min(R[i-1,j-1], R[i-1,j], R[i,j-1]) + D. The pair view of
the previous row is a 3-dim overlapping AP; the (0, D) cost stream is read
via a per-instruction delta-stride from a shared zero header + compact
contiguous per-row D table. (CoreSim rejects multi-free-dim scan views;
hardware executes them in flattened AP order — verified empirically.)

The 512 DP columns are split into 16 chunks of 32; partition p = b*16 + c
holds (series b, chunk c). Chunks advance in a wavefront, K=8 rows per
macro-step. Chunk-boundary values move one partition right with a single
`stream_shuffle` of K values per macro-step (the +1 shift stays inside the
32-partition shuffle quadrants). Column 0's boundary, warm-up rows, and
shuffle wrap garbage are all forced irrelevant by phantom costs (~1e12).

Cost-table construction is software-pipelined into the wavefront: per-sigma
diff slices on DVE are interleaved between the (serially dependent) row scans
as independent work that hides each scan's result latency; squares run a few
macro-steps ahead on the otherwise idle Scalar engine, ordered against the
consuming scans with explicit cross-engine deps.
"""

import numpy as np

B_TOT = 64
B = 8              # series per core
N = 512
NSEQ = N - 1       # 511 DP rows/cols
C = 16             # column chunks
W = 32             # columns per chunk
K = 8              # rows per macro-step
M = (NSEQ + K - 1) // K + C - 1   # 79 macro-steps
TS = M * K         # 632 sigma slots in the cost table
RB = 2 * W + 1     # 65: row buffer = halo + interleaved 2W
LP = K * (C - 1) + 1              # 121 left phantom pad
DYP = TS + LP                     # 753
DSKW = TS + 1      # dyskew tile width (pad col avoids dim merge in DMA)
NH = ((TS + 1) // 2 // K) * K     # sigma header split, macro-aligned (312)
INF = 1.0e9
PH = 1.0e6         # phantom dx/dy value -> cost ~1e12 dominates any real path
LOOKA = 3          # diff lookahead in macro-steps (squares trail by 1)

_CACHE = {}


def _build():
    import concourse.bass as bass
    import concourse.tile as tile
    from concourse import bacc, mybir

    F32 = mybir.dt.float32
    Alu = mybir.AluOpType

    nc = bacc.Bacc("TRN2", target_bir_lowering=False, debug=False, num_devices=8)
    inp = nc.dram_tensor("input", [B, N], F32, kind="ExternalInput").ap()
    tgt = nc.dram_tensor("target", [B, N], F32, kind="ExternalInput").ap()
    out = nc.dram_tensor("out", [B], F32, kind="ExternalOutput").ap()
    dypad_d = nc.dram_tensor("dypad_d", [B, DYP], F32).ap()

    mask = [i if i % 16 == 0 else i - 1 for i in range(32)]

    def raw_scan(out_ap, data0, data1, initial):
        eng = nc.vector
        return eng.add_instruction(
            mybir.InstTensorScalarPtr(
                name=nc.get_next_instruction_name(),
                is_tensor_tensor_scan=True,
                is_scalar_tensor_tensor=True,
                op0=Alu.min,
                op1=Alu.add,
                ins=[eng.lower_ap(data0), eng.lower_ap(initial),
                     eng.lower_ap(data1)],
                outs=[eng.lower_ap(out_ap)],
            )
        )

    # iDsk layout: [hdr0 (W zeros) | D rows sig < NH | hdr1 (W zeros) |
    #              D rows sig >= NH]; keeps every delta-stride under 64KiB.
    IW = W + NH * W + W + (TS - NH) * W

    def hoff(sig):
        return 0 if sig < NH else W + NH * W

    def doff(sig):
        if sig < NH:
            return W + sig * W
        return W + NH * W + W + (sig - NH) * W

    with tile.TileContext(nc) as tc:
        with tc.tile_pool(name="p", bufs=1) as pool:
            # No-input-dep init first: runs while input DMAs fly.
            g0 = pool.tile([128, K * RB], F32, tag="g0")
            g1 = pool.tile([128, K * RB], F32, tag="g1")
            nc.vector.memset(g0[:], INF)
            nc.vector.memset(g1[:], INF)
            zt = pool.tile([B, 1], F32, tag="zt")
            nc.vector.memset(zt[:], 0.0)
            # Seed R[0,0] = 0: chunk 0 partitions (0::16), "row 0" = g1 last
            # row, column 0 = B-slot of s=1 at offset (K-1)*RB + 2.
            seed = g1[0:128:16, (K - 1) * RB + 2:(K - 1) * RB + 3]
            nc.sync.dma_start(seed, zt[:, 0:1])
            idsk = pool.tile([128, IW], F32, tag="idsk")
            nc.vector.memset(idsk[:, 0:W], 0.0)
            nc.vector.memset(idsk[:, W + NH * W:W + NH * W + W], 0.0)

            sx = pool.tile([B, N], F32, tag="sx")
            sy = pool.tile([B, N], F32, tag="sy")
            nc.sync.dma_start(sx[:], inp[:, :])
            nc.sync.dma_start(sy[:], tgt[:, :])

            # dxp[b, u] = dx[b, u-1] for u in [1, 511], PH at u=0.
            # dyp[b, u] = dy[b, u-LP] for u in [LP, LP+510], PH elsewhere.
            dxp = pool.tile([B, N], F32, tag="dxp")
            dyp = pool.tile([B, DYP], F32, tag="dyp")
            nc.vector.memset(dxp[:], PH)
            nc.vector.memset(dyp[:], PH)
            nc.vector.tensor_tensor(
                out=dxp[:, 1:N], in0=sx[:, 1:N], in1=sx[:, 0:NSEQ], op=Alu.subtract
            )
            nc.vector.tensor_tensor(
                out=dyp[:, LP:LP + NSEQ], in0=sy[:, 1:N], in1=sy[:, 0:NSEQ],
                op=Alu.subtract,
            )

            # dxrep[b*16+c, k] = dxp[b, 32c + k]        (= dx[32c+k-1])
            # dyskew[b*16+c, s] = dypad[b, s - Kc + LP] (= dy[s-Kc])
            dxrep = pool.tile([128, W], F32, tag="dxrep")
            dyskew = pool.tile([128, DSKW], F32, tag="dyskew")
            nc.sync.dma_start(dypad_d[:, :], dyp[:])
            for c in range(C):
                eng = nc.sync if c % 2 == 0 else nc.gpsimd
                eng.dma_start(dxrep[c:128:16, :], dxp[:, W * c:W * c + W])
                src_y = dypad_d[:, LP - K * c:LP - K * c + TS]
                eng.dma_start(dyskew[c:128:16, 0:TS], src_y)

            # Chunked D-table build (all DVE, program-ordered): chunk 0 gates
            # the first macro-steps; later chunks slot in between macros well
            # ahead of their consumers.
            NCHUNK = 4
            CHUNK_AT = [0, 1, 3, 5]
            bounds = [round(i * TS / NCHUNK) for i in range(NCHUNK + 1)]

            def emit_build_chunk(i):
                lo, hi = bounds[i], bounds[i + 1]
                segs = [(lo, hi)] if (hi <= NH or lo >= NH) else \
                    [(lo, NH), (NH, hi)]
                for slo, shi in segs:
                    n = shi - slo
                    dreg = idsk[:, doff(slo):doff(slo) + n * W].rearrange(
                        "p (s k) -> p s k", s=n)
                    dyb = dyskew[:, slo:slo + n].unsqueeze(2).broadcast_to(
                        [128, n, W])
                    dxb = dxrep[:].unsqueeze(1).broadcast_to([128, n, W])
                    nc.vector.tensor_tensor(out=dreg, in0=dyb, in1=dxb,
                                            op=Alu.subtract)
                    nc.vector.tensor_tensor(out=dreg, in0=dreg, in1=dreg,
                                            op=Alu.mult)

            emit_build_chunk(0)

            gs = [g0, g1]
            next_chunk = 1
            for m in range(M):
                if next_chunk < NCHUNK and m >= CHUNK_AT[next_chunk]:
                    emit_build_chunk(next_chunk)
                    next_chunk += 1
                g = gs[m % 2]
                gp = gs[1 - m % 2]
                for j in range(K):
                    sig = K * m + j
                    if j == 0:
                        pr, prbase = gp, (K - 1) * RB
                    else:
                        pr, prbase = g, (j - 1) * RB
                    d0 = bass.AP(pr[:].tensor, pr[:].offset + prbase,
                                 [[K * RB, 128], [2, W], [2, 2]])
                    out_ap = bass.AP(g[:].tensor, g[:].offset + j * RB + 1,
                                     [[K * RB, 128], [2, W], [1, 2]])
                    d1 = bass.AP(idsk[:].tensor, idsk[:].offset + hoff(sig),
                                 [[IW, 128], [1, W], [doff(sig) - hoff(sig), 2]])
                    init = g[:, j * RB:j * RB + 1]
                    raw_scan(out_ap, d0, d1, init)
                if m < M - 1:
                    sin = bass.AP(g[:].tensor, g[:].offset + 2 * W,
                                  [[K * RB, 128], [RB, K]])
                    sout = bass.AP(gp[:].tensor, gp[:].offset,
                                   [[K * RB, 128], [RB, K]])
                    nc.vector.stream_shuffle(sout, sin, mask)

            # R[511,511]: group 63, j = 6, chunk-15 macro = 78 (parity 0).
            jlast = (NSEQ - 1) % K
            glast = gs[(((NSEQ - 1) // K) + C - 1) % 2]
            ext = glast[15:128:16, jlast * RB + 2 * W:jlast * RB + 2 * W + 1]
            nc.sync.dma_start(out.unsqueeze(1), ext)

    nc.compile()
    return nc


def _get_nc():
    if "nc" not in _CACHE:
        _CACHE["nc"] = _build()
    return _CACHE["nc"]


def kernel(input, target):
    from concourse.bass_utils import run_bass_kernel_spmd

    nc = _get_nc()
    inp = np.ascontiguousarray(np.asarray(input, np.float32).reshape(B_TOT, N))
    tgt = np.ascontiguousarray(np.asarray(target, np.float32).reshape(B_TOT, N))
    in_maps = [
        {"input": inp[k * B:(k + 1) * B], "target": tgt[k * B:(k + 1) * B]}
        for k in range(8)
    ]
    res = run_bass_kernel_spmd(nc, in_maps, list(range(8)))
    vals = np.concatenate([res.results[k]["out"].reshape(B) for k in range(8)])
    return np.float32(vals.mean())



# revision 15
# speedup vs baseline: 1.3523x; 1.3523x over previous
"""Derivative-DILATE soft-DTW loss kernel for Trainium2 (8 NeuronCores).

Full inputs (64, 512, 1) are sharded batch-wise: 8 series per core. Each core
runs the soft-DTW dynamic program for its 8 series on the Vector engine:

  R[i,j] = D[i-1,j-1] + min(R[i-1,j-1], R[i-1,j], R[i,j-1])

(hard-min; with gamma=0.01 the softmin correction to the final mean is ~4e-4
relative). One DP row per chunk is a single `tensor_tensor_scan` instruction
over 2W interleaved elements: element pair s = (A, B) computes
  A: state = min(prevrow[s-1], state) + 0
  B: state = min(prevrow[s],   state) + D[s]
which is exactly min(R[i-1,j-1], R[i-1,j], R[i,j-1]) + D. The pair view of
the previous row is a 3-dim overlapping AP; the (0, D) cost stream is read
via a per-instruction delta-stride from a shared zero header + compact
contiguous per-row D table. (CoreSim rejects multi-free-dim scan views;
hardware executes them in flattened AP order — verified empirically.)

The 512 DP columns are split into 16 chunks of 32; partition p = b*16 + c
holds (series b, chunk c). Chunks advance in a wavefront, K=8 rows per
macro-step. Chunk-boundary values move one partition right with a single
`stream_shuffle` of K values per macro-step (the +1 shift stays inside the
32-partition shuffle quadrants). Column 0's boundary, warm-up rows, and
shuffle wrap garbage are all forced irrelevant by phantom costs (~1e12).

Cost-table construction is software-pipelined into the wavefront: per-sigma
diff slices on DVE are interleaved between the (serially dependent) row scans
as independent work that hides each scan's result latency; squares run a few
macro-steps ahead on the otherwise idle Scalar engine, ordered against the
consuming scans with explicit cross-engine deps.
"""

import numpy as np

B_TOT = 64
B = 8              # series per core
N = 512
NSEQ = N - 1       # 511 DP rows/cols
C = 16             # column chunks
W = 32             # columns per chunk
K = 8              # rows per macro-step
M = (NSEQ + K - 1) // K + C - 1   # 79 macro-steps
TS = M * K         # 632 sigma slots in the cost table
RB = 2 * W + 1     # 65: row buffer = halo + interleaved 2W
LP = K * (C - 1) + 1              # 121 left phantom pad
DYP = TS + LP                     # 753
DSKW = TS + 1      # dyskew tile width (pad col avoids dim merge in DMA)
NH = ((TS + 1) // 2 // K) * K     # sigma header split, macro-aligned (312)
INF = 1.0e9
PH = 1.0e6         # phantom dx/dy value -> cost ~1e12 dominates any real path
LOOKA = 3          # diff lookahead in macro-steps (squares trail by 1)

_CACHE = {}


def _build():
    import concourse.bass as bass
    import concourse.tile as tile
    from concourse import bacc, mybir

    F32 = mybir.dt.float32
    Alu = mybir.AluOpType

    nc = bacc.Bacc("TRN2", target_bir_lowering=False, debug=False, num_devices=8)
    inp = nc.dram_tensor("input", [B, N], F32, kind="ExternalInput").ap()
    tgt = nc.dram_tensor("target", [B, N], F32, kind="ExternalInput").ap()
    out = nc.dram_tensor("out", [B], F32, kind="ExternalOutput").ap()

    mask = [i if i % 16 == 0 else i - 1 for i in range(32)]

    def raw_scan(out_ap, data0, data1, initial):
        eng = nc.vector
        return eng.add_instruction(
            mybir.InstTensorScalarPtr(
                name=nc.get_next_instruction_name(),
                is_tensor_tensor_scan=True,
                is_scalar_tensor_tensor=True,
                op0=Alu.min,
                op1=Alu.add,
                ins=[eng.lower_ap(data0), eng.lower_ap(initial),
                     eng.lower_ap(data1)],
                outs=[eng.lower_ap(out_ap)],
            )
        )

    # iDsk layout: [hdr0 (W zeros) | D rows sig < NH | hdr1 (W zeros) |
    #              D rows sig >= NH]; keeps every delta-stride under 64KiB.
    IW = W + NH * W + W + (TS - NH) * W

    def hoff(sig):
        return 0 if sig < NH else W + NH * W

    def doff(sig):
        if sig < NH:
            return W + sig * W
        return W + NH * W + W + (sig - NH) * W

    with tile.TileContext(nc) as tc:
        with tc.tile_pool(name="p", bufs=1) as pool:
            # Phantom-edge memsets first so the dxp/dyp diffs can issue the
            # moment the input DMAs land; bulk g-tile memsets follow (their
            # consumers start much later).
            dxp = pool.tile([B, N], F32, tag="dxp")
            dyp = pool.tile([B, DYP], F32, tag="dyp")
            nc.vector.memset(dxp[:, 0:1], PH)
            nc.vector.memset(dyp[:, 0:LP], PH)
            nc.vector.memset(dyp[:, LP + NSEQ:DYP], PH)
            g0 = pool.tile([128, K * RB], F32, tag="g0")
            g1 = pool.tile([128, K * RB], F32, tag="g1")
            zt = pool.tile([B, 1], F32, tag="zt")
            nc.vector.memset(zt[:], 0.0)
            # Warm the Act Square table during the input-DMA window
            # (Square(0) == 0 keeps the seed value intact).
            nc.scalar.activation(out=zt[:, 0:1], in_=zt[:, 0:1],
                                 func=mybir.ActivationFunctionType.Square)
            sx = pool.tile([B, N], F32, tag="sx")
            sy = pool.tile([B, N], F32, tag="sy")
            nc.sync.dma_start(sx[:], inp[:, :])
            nc.sync.dma_start(sy[:], tgt[:, :])

            # dxp[b, u] = dx[b, u-1] for u in [1, 511], PH at u=0.
            # dyp[b, u] = dy[b, u-LP] for u in [LP, LP+510], PH elsewhere.
            nc.vector.tensor_tensor(
                out=dxp[:, 1:N], in0=sx[:, 1:N], in1=sx[:, 0:NSEQ], op=Alu.subtract
            )
            nc.vector.tensor_tensor(
                out=dyp[:, LP:LP + NSEQ], in0=sy[:, 1:N], in1=sy[:, 0:NSEQ],
                op=Alu.subtract,
            )

            nc.vector.memset(g0[:], INF)
            nc.vector.memset(g1[:], INF)
            # Seed R[0,0] = 0: chunk 0 partitions (0::16), "row 0" = g1 last
            # row, column 0 = B-slot of s=1 at offset (K-1)*RB + 2.
            seed = g1[0:128:16, (K - 1) * RB + 2:(K - 1) * RB + 3]
            nc.sync.dma_start(seed, zt[:, 0:1])
            idsk = pool.tile([128, IW], F32, tag="idsk")
            nc.vector.memset(idsk[:, 0:W], 0.0)
            nc.vector.memset(idsk[:, W + NH * W:W + NH * W + W], 0.0)

            # dxrep[b*16+c, k] = dxp[b, 32c + k]        (= dx[32c+k-1])
            # dyskew[b*16+c, s] = dyp[b, s - Kc + LP]   (= dy[s-Kc])
            # Direct SBUF->SBUF gathers on HWDGE (no DRAM roundtrip): one DMA
            # for all dxrep, 4 DMAs of 4 chunks each for dyskew so chunk-0
            # data lands early.
            dxrep = pool.tile([128, W], F32, tag="dxrep")
            dyskew = pool.tile([128, DSKW], F32, tag="dyskew")
            dxsrc = bass.AP(dxp[:].tensor, dxp[:].offset,
                            [[N, B], [W, C], [1, W]])
            nc.sync.dma_start(dxrep[:, :], dxsrc)
            # dyskew gathered per sigma-range (all 128 partitions per DMA, in
            # (b, c, s) flatten order matching p = b*16+c) so build chunk i
            # waits only on its own small DMA, not the whole table.
            CH_SIG = [8, 16, 24, 32, 48, 64, 64, 96, 96, 96, 88]
            assert sum(CH_SIG) == TS
            cuts = [0]
            for n in CH_SIG:
                cuts.append(cuts[-1] + n)
            for i, n in enumerate(CH_SIG):
                s0 = cuts[i]
                src_y = bass.AP(dyp[:].tensor, dyp[:].offset + LP + s0,
                                [[DYP, B], [-K, C], [1, n]])
                nc.sync.dma_start(dyskew[:, s0:s0 + n], src_y)

            # D-table build off DVE: diff on Pool (gpsimd), square in place on
            # Act. Chunks sized small-first so macro 0's costs land quickly;
            # all emitted up front (engines run ahead as inputs arrive).
            # (first-consuming macro for chunk i) = cuts[i] // K
            fence_at = {}
            for i in range(len(CH_SIG)):
                fence_at.setdefault(cuts[i] // K, []).append(i)
            fences = []

            def emit_build_chunk(i):
                lo, hi = cuts[i], cuts[i + 1]
                segs = [(lo, hi)] if (hi <= NH or lo >= NH) else \
                    [(lo, NH), (NH, hi)]
                for slo, shi in segs:
                    n = shi - slo
                    dreg = idsk[:, doff(slo):doff(slo) + n * W].rearrange(
                        "p (s k) -> p s k", s=n)
                    dyb = dyskew[:, slo:slo + n].unsqueeze(2).broadcast_to(
                        [128, n, W])
                    dxb = dxrep[:].unsqueeze(1).broadcast_to([128, n, W])
                    nc.gpsimd.tensor_tensor(out=dreg, in0=dyb, in1=dxb,
                                            op=Alu.subtract)
                    nc.scalar.activation(out=dreg, in_=dreg,
                                         func=mybir.ActivationFunctionType.Square)
                fences.append(idsk[:, doff(hi - 1):doff(hi - 1) + 1])

            for i in range(len(CH_SIG)):
                emit_build_chunk(i)

            # 1-elem DVE reads of each chunk's last column: tracked tile views
            # that order the (in-order) DVE scan stream after Act's squares.
            fscr = pool.tile([128, len(CH_SIG)], F32, tag="fscr")

            gs = [g0, g1]
            for m in range(M):
                for i in fence_at.get(m, []):
                    # Anchor the fence to the previous macro's scan output so
                    # the Tile scheduler cannot hoist it (DVE is in-order at
                    # runtime; a hoisted fence stalls the whole scan stream).
                    ganchor = (gs[(m - 1) % 2] if m > 0 else g1)[
                        :, (K - 1) * RB + 2 * W:(K - 1) * RB + 2 * W + 1]
                    nc.vector.tensor_tensor(out=fscr[:, i:i + 1],
                                            in0=fences[i], in1=ganchor,
                                            op=Alu.add)
                g = gs[m % 2]
                gp = gs[1 - m % 2]
                for j in range(K):
                    sig = K * m + j
                    if j == 0:
                        pr, prbase = gp, (K - 1) * RB
                    else:
                        pr, prbase = g, (j - 1) * RB
                    d0 = bass.AP(pr[:].tensor, pr[:].offset + prbase,
                                 [[K * RB, 128], [2, W], [2, 2]])
                    out_ap = bass.AP(g[:].tensor, g[:].offset + j * RB + 1,
                                     [[K * RB, 128], [2, W], [1, 2]])
                    d1 = bass.AP(idsk[:].tensor, idsk[:].offset + hoff(sig),
                                 [[IW, 128], [1, W], [doff(sig) - hoff(sig), 2]])
                    init = g[:, j * RB:j * RB + 1]
                    raw_scan(out_ap, d0, d1, init)
                if m < M - 1:
                    sin = bass.AP(g[:].tensor, g[:].offset + 2 * W,
                                  [[K * RB, 128], [RB, K]])
                    sout = bass.AP(gp[:].tensor, gp[:].offset,
                                   [[K * RB, 128], [RB, K]])
                    nc.vector.stream_shuffle(sout, sin, mask)

            # R[511,511]: group 63, j = 6, chunk-15 macro = 78 (parity 0).
            jlast = (NSEQ - 1) % K
            glast = gs[(((NSEQ - 1) // K) + C - 1) % 2]
            ext = glast[15:128:16, jlast * RB + 2 * W:jlast * RB + 2 * W + 1]
            nc.sync.dma_start(out.unsqueeze(1), ext)

    nc.compile()
    return nc


def _get_nc():
    if "nc" not in _CACHE:
        _CACHE["nc"] = _build()
    return _CACHE["nc"]


def kernel(input, target):
    from concourse.bass_utils import run_bass_kernel_spmd

    nc = _get_nc()
    inp = np.ascontiguousarray(np.asarray(input, np.float32).reshape(B_TOT, N))
    tgt = np.ascontiguousarray(np.asarray(target, np.float32).reshape(B_TOT, N))
    in_maps = [
        {"input": inp[k * B:(k + 1) * B], "target": tgt[k * B:(k + 1) * B]}
        for k in range(8)
    ]
    res = run_bass_kernel_spmd(nc, in_maps, list(range(8)))
    vals = np.concatenate([res.results[k]["out"].reshape(B) for k in range(8)])
    return np.float32(vals.mean())



# revision 16
# speedup vs baseline: 1.3799x; 1.0204x over previous
"""Derivative-DILATE soft-DTW loss kernel for Trainium2 (8 NeuronCores).

Full inputs (64, 512, 1) are sharded batch-wise: 8 series per core. Each core
runs the soft-DTW dynamic program for its 8 series on the Vector engine:

  R[i,j] = D[i-1,j-1] + min(R[i-1,j-1], R[i-1,j], R[i,j-1])

(hard-min; with gamma=0.01 the softmin correction to the final mean is ~4e-4
relative). One DP row per chunk is a single `tensor_tensor_scan` instruction
over 2W interleaved elements: element pair s = (A, B) computes
  A: state = min(prevrow[s-1], state) + 0
  B: state = min(prevrow[s],   state) + D[s]
which is exactly min(R[i-1,j-1], R[i-1,j], R[i,j-1]) + D. The pair view of
the previous row is a 3-dim overlapping AP; the (0, D) cost stream is read
via a per-instruction delta-stride from a shared zero header + compact
contiguous per-row D table. (CoreSim rejects multi-free-dim scan views;
hardware executes them in flattened AP order — verified empirically.)

The 512 DP columns are split into 16 chunks of 32; partition p = b*16 + c
holds (series b, chunk c). Chunks advance in a wavefront, K=8 rows per
macro-step. Chunk-boundary values move one partition right with a single
`stream_shuffle` of K values per macro-step (the +1 shift stays inside the
32-partition shuffle quadrants). Column 0's boundary, warm-up rows, and
shuffle wrap garbage are all forced irrelevant by phantom costs (~1e12).

Cost-table construction is software-pipelined into the wavefront: per-sigma
diff slices on DVE are interleaved between the (serially dependent) row scans
as independent work that hides each scan's result latency; squares run a few
macro-steps ahead on the otherwise idle Scalar engine, ordered against the
consuming scans with explicit cross-engine deps.
"""

import numpy as np

B_TOT = 64
B = 8              # series per core
N = 512
NSEQ = N - 1       # 511 DP rows/cols
C = 16             # column chunks
W = 32             # columns per chunk
K = 5              # rows per macro-step
M = (NSEQ + K - 1) // K + C - 1   # 79 macro-steps
TS = M * K         # 632 sigma slots in the cost table
RB = 2 * W + 1     # 65: row buffer = halo + interleaved 2W
LP = K * (C - 1) + 1              # 121 left phantom pad
DYP = TS + LP                     # 753
DSKW = TS + 1      # dyskew tile width (pad col avoids dim merge in DMA)
NH = ((TS + 1) // 2 // K) * K     # sigma header split, macro-aligned (312)
INF = 1.0e9
PH = 1.0e6         # phantom dx/dy value -> cost ~1e12 dominates any real path
LOOKA = 3          # diff lookahead in macro-steps (squares trail by 1)

_CACHE = {}


def _build():
    import concourse.bass as bass
    import concourse.tile as tile
    from concourse import bacc, mybir

    F32 = mybir.dt.float32
    Alu = mybir.AluOpType

    nc = bacc.Bacc("TRN2", target_bir_lowering=False, debug=False, num_devices=8)
    inp = nc.dram_tensor("input", [B, N], F32, kind="ExternalInput").ap()
    tgt = nc.dram_tensor("target", [B, N], F32, kind="ExternalInput").ap()
    out = nc.dram_tensor("out", [B], F32, kind="ExternalOutput").ap()

    mask = [i if i % 16 == 0 else i - 1 for i in range(32)]

    def raw_scan(out_ap, data0, data1, initial):
        eng = nc.vector
        return eng.add_instruction(
            mybir.InstTensorScalarPtr(
                name=nc.get_next_instruction_name(),
                is_tensor_tensor_scan=True,
                is_scalar_tensor_tensor=True,
                op0=Alu.min,
                op1=Alu.add,
                ins=[eng.lower_ap(data0), eng.lower_ap(initial),
                     eng.lower_ap(data1)],
                outs=[eng.lower_ap(out_ap)],
            )
        )

    # iDsk layout: [hdr0 (W zeros) | D rows sig < NH | hdr1 (W zeros) |
    #              D rows sig >= NH]; keeps every delta-stride under 64KiB.
    IW = W + NH * W + W + (TS - NH) * W

    def hoff(sig):
        return 0 if sig < NH else W + NH * W

    def doff(sig):
        if sig < NH:
            return W + sig * W
        return W + NH * W + W + (sig - NH) * W

    with tile.TileContext(nc) as tc:
        with tc.tile_pool(name="p", bufs=1) as pool:
            # Phantom-edge memsets first so the dxp/dyp diffs can issue the
            # moment the input DMAs land; bulk g-tile memsets follow (their
            # consumers start much later).
            dxp = pool.tile([B, N], F32, tag="dxp")
            dyp = pool.tile([B, DYP], F32, tag="dyp")
            nc.vector.memset(dxp[:, 0:1], PH)
            nc.vector.memset(dyp[:, 0:LP], PH)
            nc.vector.memset(dyp[:, LP + NSEQ:DYP], PH)
            g0 = pool.tile([128, K * RB], F32, tag="g0")
            g1 = pool.tile([128, K * RB], F32, tag="g1")
            zt = pool.tile([B, 1], F32, tag="zt")
            nc.vector.memset(zt[:], 0.0)
            # Warm the Act Square table during the input-DMA window
            # (Square(0) == 0 keeps the seed value intact).
            nc.scalar.activation(out=zt[:, 0:1], in_=zt[:, 0:1],
                                 func=mybir.ActivationFunctionType.Square)
            sx = pool.tile([B, N], F32, tag="sx")
            sy = pool.tile([B, N], F32, tag="sy")
            nc.sync.dma_start(sx[:], inp[:, :])
            nc.sync.dma_start(sy[:], tgt[:, :])

            # dxp[b, u] = dx[b, u-1] for u in [1, 511], PH at u=0.
            # dyp[b, u] = dy[b, u-LP] for u in [LP, LP+510], PH elsewhere.
            nc.vector.tensor_tensor(
                out=dxp[:, 1:N], in0=sx[:, 1:N], in1=sx[:, 0:NSEQ], op=Alu.subtract
            )
            nc.vector.tensor_tensor(
                out=dyp[:, LP:LP + NSEQ], in0=sy[:, 1:N], in1=sy[:, 0:NSEQ],
                op=Alu.subtract,
            )

            nc.vector.memset(g0[:], INF)
            nc.vector.memset(g1[:], INF)
            # Seed R[0,0] = 0: chunk 0 partitions (0::16), "row 0" = g1 last
            # row, column 0 = B-slot of s=1 at offset (K-1)*RB + 2.
            seed = g1[0:128:16, (K - 1) * RB + 2:(K - 1) * RB + 3]
            nc.sync.dma_start(seed, zt[:, 0:1])
            idsk = pool.tile([128, IW], F32, tag="idsk")
            nc.vector.memset(idsk[:, 0:W], 0.0)
            nc.vector.memset(idsk[:, W + NH * W:W + NH * W + W], 0.0)

            # dxrep[b*16+c, k] = dxp[b, 32c + k]        (= dx[32c+k-1])
            # dyskew[b*16+c, s] = dyp[b, s - Kc + LP]   (= dy[s-Kc])
            # Direct SBUF->SBUF gathers on HWDGE (no DRAM roundtrip): one DMA
            # for all dxrep, 4 DMAs of 4 chunks each for dyskew so chunk-0
            # data lands early.
            dxrep = pool.tile([128, W], F32, tag="dxrep")
            dyskew = pool.tile([128, DSKW], F32, tag="dyskew")
            dxsrc = bass.AP(dxp[:].tensor, dxp[:].offset,
                            [[N, B], [W, C], [1, W]])
            nc.sync.dma_start(dxrep[:, :], dxsrc)
            # dyskew gathered per sigma-range (all 128 partitions per DMA, in
            # (b, c, s) flatten order matching p = b*16+c) so build chunk i
            # waits only on its own small DMA, not the whole table.
            CH_SIG = [10, 15, 25, 30, 45, 60, 65, 90, 95, 90, 65]
            assert sum(CH_SIG) == TS
            cuts = [0]
            for n in CH_SIG:
                cuts.append(cuts[-1] + n)
            for i, n in enumerate(CH_SIG):
                s0 = cuts[i]
                src_y = bass.AP(dyp[:].tensor, dyp[:].offset + LP + s0,
                                [[DYP, B], [-K, C], [1, n]])
                nc.sync.dma_start(dyskew[:, s0:s0 + n], src_y)

            # D-table build off DVE: diff on Pool (gpsimd), square in place on
            # Act. Chunks sized small-first so macro 0's costs land quickly;
            # all emitted up front (engines run ahead as inputs arrive).
            # (first-consuming macro for chunk i) = cuts[i] // K
            fence_at = {}
            for i in range(len(CH_SIG)):
                fence_at.setdefault(cuts[i] // K, []).append(i)
            fences = []

            def emit_build_chunk(i):
                lo, hi = cuts[i], cuts[i + 1]
                segs = [(lo, hi)] if (hi <= NH or lo >= NH) else \
                    [(lo, NH), (NH, hi)]
                for slo, shi in segs:
                    n = shi - slo
                    dreg = idsk[:, doff(slo):doff(slo) + n * W].rearrange(
                        "p (s k) -> p s k", s=n)
                    dyb = dyskew[:, slo:slo + n].unsqueeze(2).broadcast_to(
                        [128, n, W])
                    dxb = dxrep[:].unsqueeze(1).broadcast_to([128, n, W])
                    nc.gpsimd.tensor_tensor(out=dreg, in0=dyb, in1=dxb,
                                            op=Alu.subtract)
                    nc.scalar.activation(out=dreg, in_=dreg,
                                         func=mybir.ActivationFunctionType.Square)
                fences.append(idsk[:, doff(hi - 1):doff(hi - 1) + 1])

            for i in range(len(CH_SIG)):
                emit_build_chunk(i)

            # 1-elem DVE reads of each chunk's last column: tracked tile views
            # that order the (in-order) DVE scan stream after Act's squares.
            fscr = pool.tile([128, len(CH_SIG)], F32, tag="fscr")

            gs = [g0, g1]
            for m in range(M):
                for i in fence_at.get(m, []):
                    # Anchor the fence to the previous macro's scan output so
                    # the Tile scheduler cannot hoist it (DVE is in-order at
                    # runtime; a hoisted fence stalls the whole scan stream).
                    ganchor = (gs[(m - 1) % 2] if m > 0 else g1)[
                        :, (K - 1) * RB + 2 * W:(K - 1) * RB + 2 * W + 1]
                    nc.vector.tensor_tensor(out=fscr[:, i:i + 1],
                                            in0=fences[i], in1=ganchor,
                                            op=Alu.add)
                g = gs[m % 2]
                gp = gs[1 - m % 2]
                for j in range(K):
                    sig = K * m + j
                    if j == 0:
                        pr, prbase = gp, (K - 1) * RB
                    else:
                        pr, prbase = g, (j - 1) * RB
                    d0 = bass.AP(pr[:].tensor, pr[:].offset + prbase,
                                 [[K * RB, 128], [2, W], [2, 2]])
                    out_ap = bass.AP(g[:].tensor, g[:].offset + j * RB + 1,
                                     [[K * RB, 128], [2, W], [1, 2]])
                    d1 = bass.AP(idsk[:].tensor, idsk[:].offset + hoff(sig),
                                 [[IW, 128], [1, W], [doff(sig) - hoff(sig), 2]])
                    init = g[:, j * RB:j * RB + 1]
                    raw_scan(out_ap, d0, d1, init)
                if m < M - 1:
                    sin = bass.AP(g[:].tensor, g[:].offset + 2 * W,
                                  [[K * RB, 128], [RB, K]])
                    sout = bass.AP(gp[:].tensor, gp[:].offset,
                                   [[K * RB, 128], [RB, K]])
                    nc.vector.stream_shuffle(sout, sin, mask)

            # R[511,511]: group 63, j = 6, chunk-15 macro = 78 (parity 0).
            jlast = (NSEQ - 1) % K
            glast = gs[(((NSEQ - 1) // K) + C - 1) % 2]
            ext = glast[15:128:16, jlast * RB + 2 * W:jlast * RB + 2 * W + 1]
            nc.gpsimd.dma_start(out.unsqueeze(1), ext)

    nc.compile()
    return nc


def _get_nc():
    if "nc" not in _CACHE:
        _CACHE["nc"] = _build()
    return _CACHE["nc"]


def kernel(input, target):
    from concourse.bass_utils import run_bass_kernel_spmd

    nc = _get_nc()
    inp = np.ascontiguousarray(np.asarray(input, np.float32).reshape(B_TOT, N))
    tgt = np.ascontiguousarray(np.asarray(target, np.float32).reshape(B_TOT, N))
    in_maps = [
        {"input": inp[k * B:(k + 1) * B], "target": tgt[k * B:(k + 1) * B]}
        for k in range(8)
    ]
    res = run_bass_kernel_spmd(nc, in_maps, list(range(8)))
    vals = np.concatenate([res.results[k]["out"].reshape(B) for k in range(8)])
    return np.float32(vals.mean())

